# revision 1
# baseline (speedup 1.0000x reference)
"""Multi-head causal attention (B=4,S=1024,D=768,H=12,Dh=64) on 8 trn2 cores.

Sharding: core c handles batch b=c//2 and the 6 heads hs=(c%2)*6 .. hs+6
(head-axis tensor parallel x batch parallel; 8 cores = 4 batches x 2 head-halves).

Per-core on-chip dataflow (bf16 matmul operands, fp32 PSUM accumulation):
  xT [768,1024] (host-pretransposed bf16), W{q,k,v} stacked [768, 384] bf16
  qT/kT = W-chunk.T(lhsT) @ xT    -> [64,1024] per head (transposed layout)
  v     = xT-chunk.T @ Wv          -> [1024, 6*65] per t-chunk (65th col = ones)
  scoresT[t,s] tiles = kT-chunk(lhsT) x qT(rhs); fully-causal tiles skipped,
  diag tiles masked by accumulating identity @ (-30000 strict-lower-tri) in PSUM
  exp via ScalarE Exp(scale=1/8) straight from PSUM into a flat bf16 SBUF buffer
  ctxT_aug[65, s] = sum_j v_aug_j(lhsT) @ expT_j  (row 64 = softmax denominator)
  y_aug[h, 0:65, s] DMA'd out fp32; host divides by denominators + transposes.
"""

import threading
from contextlib import ExitStack

import ml_dtypes
import numpy as np

import concourse.bass as bass
import concourse.tile as tile
from concourse import bacc, mybir
from concourse.bass_utils import run_bass_kernel_spmd

B, S, D, H, DH = 4, 1024, 768, 12, 64
NCORES = 8
HL = H // 2          # 6 local heads per core
KC = D // 128        # 6 contraction chunks
NPAIR = HL // 2      # head pairs for qk projection
F32 = mybir.dt.float32
BF16 = mybir.dt.bfloat16
MASK_VAL = -30000.0


def _attn_groups():
    """Chunk table for one head's scoresT, packed into [128,1024] PSUM groups.

    A chunk (j, c) is the scoresT tile for t-chunk j (rows j*128..j*128+128)
    and s-range [s0, s0+w) inside output half c (s in [512c, 512c+512)).
    Only causal-relevant chunks exist. `diag` chunks need the triangular mask
    added to their first 128 columns. `ps_off` is the column offset inside the
    group's PSUM tile (each chunk stays inside one 512-col PSUM bank);
    `off` is the offset in the per-head flat exp buffer.
    """
    def chunk(j, c, ps_off):
        s0 = max(512 * c, 128 * j)
        w = 512 * (c + 1) - s0
        return dict(j=j, c=c, s0=s0, w=w, diag=(s0 == 128 * j), ps_off=ps_off)

    groups = [
        [chunk(0, 1, 0), chunk(0, 0, 512)],
        [chunk(1, 1, 0), chunk(1, 0, 512), chunk(7, 1, 896)],
        [chunk(2, 1, 0), chunk(2, 0, 512), chunk(6, 1, 768)],
        [chunk(3, 1, 0), chunk(3, 0, 512), chunk(5, 1, 640)],
        [chunk(4, 1, 0)],
    ]
    base = 0
    for g in groups:
        for ch in g:
            ch["off"] = base + ch["ps_off"]
        g_w = max(ch["ps_off"] + ch["w"] for ch in g)
        base += g_w
    total = base  # 4608
    return groups, total


def _emit_kernel(ctx: ExitStack, tc: tile.TileContext, xT, wq, wk, wv, im, y):
    nc = tc.nc
    groups, exp_cols = _attn_groups()

    # identity + 0/1 causal mask arrive as a tiny host input (generating them
    # on GpSimd costs ~6us and delays the PE warm-up)
    const = ctx.enter_context(tc.tile_pool(name="const", bufs=1))
    im_sb = const.tile([128, 2, 128], BF16)
    nc.sync.dma_start(out=im_sb, in_=im[:, :, :])
    ident = im_sb[:, 0, :]
    tri01 = im_sb[:, 1, :]  # 1 where s >= t else 0

    qk_pool = ctx.enter_context(tc.tile_pool(name="qk", bufs=1))
    qT = qk_pool.tile([128, NPAIR, S], BF16)  # partitions: (h%2)*64+e, pair, s
    kT = qk_pool.tile([128, NPAIR, S], BF16)
    v_sb = qk_pool.tile([128, 8, HL * (DH + 1)], BF16)  # [t_rel, t_chunk, h*65+x]

    # pools (PSUM budget: pj 2 banks + sg 1x4 + cx 2 = 8)
    xtw = ctx.enter_context(tc.tile_pool(name="xtw", bufs=1))
    pj = ctx.enter_context(tc.tile_pool(name="pj", bufs=1, space="PSUM"))
    sg = ctx.enter_context(tc.tile_pool(name="sg", bufs=1, space="PSUM"))
    cx = ctx.enter_context(tc.tile_pool(name="cx", bufs=2, space="PSUM"))
    ex = ctx.enter_context(tc.tile_pool(name="ex", bufs=3))
    yst = ctx.enter_context(tc.tile_pool(name="yst", bufs=3))

    # PE warm-up: ~3.5us of dummy matmuls into a scratch PSUM bank so the HAM
    # clock gate opens (K=8/8, 2.4 GHz) before the real matmuls arrive.
    warm = pj.tile([128, 128], F32, tag="pjq0", name="warm")
    for i in range(28):
        nc.tensor.matmul(out=warm, lhsT=ident, rhs=tri01,
                         start=(i == 0), stop=(i == 27))

    xt = xtw.tile([128, KC, S], BF16)
    w_q = xtw.tile([128, KC, HL * DH], BF16)
    w_k = xtw.tile([128, KC, HL * DH], BF16)
    w_v = xtw.tile([128, KC, HL * DH], BF16)
    # per-chunk loads spread over four DMA queues so chunk 0 lands fast and
    # the four streams share HBM bandwidth
    for kc in range(KC):
        nc.sync.dma_start(out=xt[:, kc, :], in_=xT[kc * 128:(kc + 1) * 128, :])
        nc.scalar.dma_start(out=w_q[:, kc, :], in_=wq[kc * 128:(kc + 1) * 128, :])
        nc.scalar.dma_start(out=w_k[:, kc, :], in_=wk[kc * 128:(kc + 1) * 128, :])
        nc.gpsimd.dma_start(out=w_v[:, kc, :], in_=wv[kc * 128:(kc + 1) * 128, :])

    # ---- PE filler machinery: engines run their streams in order, so the
    # scores groups (paced by the Scalar-engine exp) must have independent
    # matmul work interleaved into the PE stream to avoid idle gaps.
    fillers = []  # list of (est_ns, emit_fn)

    def emit_fillers(budget_ns):
        while fillers and budget_ns > 0:
            est, fn = fillers.pop(0)
            fn()
            budget_ns -= est

    def proj_qk_units(pp):
        """q/k projection for pair pp as filler units (kc-outer accumulate)."""
        units = []
        for w_all, dst in ((w_q, qT), (w_k, kT)):
            pss = [pj.tile([128, 512], F32, tag=f"pjq{i}", name=f"ps{pp}{i}")
                   for i in range(2)]

            def unit(kcs, w_all=w_all, pss=pss, pp=pp, dst=dst):
                def emit():
                    for kc in kcs:
                        for i, ps in enumerate(pss):
                            nc.tensor.matmul(
                                out=ps,
                                lhsT=w_all[:, kc, pp * 128:(pp + 1) * 128],
                                rhs=xt[:, kc, i * 512:(i + 1) * 512],
                                start=(kc == 0), stop=(kc == KC - 1),
                            )
                    if kcs[-1] == KC - 1:
                        for i, ps in enumerate(pss):
                            nc.vector.tensor_copy(
                                out=dst[:, pp, i * 512:(i + 1) * 512], in_=ps)
                return emit
            units.append((900, unit([0, 1])))
            units.append((900, unit([2, 3])))
            units.append((900, unit([4, 5])))
        return units

    def proj_v_unit(j):
        def emit():
            psv = pj.tile([128, HL * DH], F32, tag=f"pjq{j % 2}", name=f"psv{j}")
            for kc in range(KC):
                nc.tensor.matmul(
                    out=psv,
                    lhsT=xt[:, kc, j * 128:(j + 1) * 128],
                    rhs=w_v[:, kc, :],
                    start=(kc == 0), stop=(kc == KC - 1),
                )
            v_dst = v_sb[:, j, :].rearrange("p (h x) -> p h x", h=HL)
            nc.vector.tensor_copy(
                out=v_dst[:, :, 0:DH],
                in_=psv.rearrange("p (h e) -> p h e", h=HL),
            )
            nc.vector.memset(v_dst[:, :, DH:DH + 1], 1.0)
        return (1100, emit)

    chunks = [ch for g in groups for ch in g]

    def ctx_unit(h, exp_pair, c):
        def emit():
            cc = sorted((ch for ch in chunks if ch["c"] == c),
                        key=lambda t: t["j"])
            pc = cx.tile([DH + 1, 512], F32, tag="cx", name=f"pc{h}{c}")
            for idx, ch in enumerate(cc):
                nc.tensor.matmul(
                    out=pc[:, ch["s0"] - 512 * c: ch["s0"] - 512 * c + ch["w"]],
                    lhsT=v_sb[:, ch["j"], :].rearrange(
                        "p (hh x) -> p hh x", hh=HL)[:, h, :],
                    rhs=exp_pair[:, h % 2, ch["off"]:ch["off"] + ch["w"]],
                    start=(idx == 0), stop=(idx == len(cc) - 1),
                )
            yt = yst.tile([DH + 1, 512], F32, tag="yst", name=f"yt{h}{c}")
            nc.vector.tensor_copy(out=yt, in_=pc)
            nc.sync.dma_start(out=y[h, :, c * 512:(c + 1) * 512], in_=yt)
        return (2200, emit)

    def scores_group(hp, g, exp_pair):
        """One scores group for both heads of pair hp into one [128,2048]
        PSUM tile (head A banks 0-1, head B banks 2-3). A/B matmuls alternate
        so their K=64 row groups (base_partition 0/64) run concurrently.
        One Exp ACT covers both heads via a strided 3D output AP. Causal
        masking of diag chunks happens afterwards on the Vector engine
        (multiply by the 0/1 triangle), keeping the PE stream pure."""
        g_w = max(ch["ps_off"] + ch["w"] for ch in g)
        ps = sg.tile([128, 2 * 1024], F32, tag="sg", name=f"sg{hp}")
        for bank in (0, 1):
            ops = [ch for ch in g if ch["ps_off"] // 512 == bank]
            for i, ch in enumerate(ops):
                first, last = (i == 0), (i == len(ops) - 1)
                for a in (0, 1):
                    half = a * 64
                    off = a * 1024 + ch["ps_off"]
                    nc.tensor.matmul(
                        out=ps[:, off:off + ch["w"]],
                        lhsT=kT[half:half + 64, hp,
                                ch["j"] * 128:(ch["j"] + 1) * 128],
                        rhs=qT[half:half + 64, hp,
                               ch["s0"]:ch["s0"] + ch["w"]],
                        start=first, stop=last,
                    )
        nc.scalar.activation(
            out=exp_pair[:, :, g[0]["off"]:g[0]["off"] + g_w],
            in_=ps.rearrange("p (h b) -> p h b", h=2)[:, :, 0:g_w],
            func=mybir.ActivationFunctionType.Exp,
            scale=1.0 / np.sqrt(DH),
        )
        for ch in g:
            if ch["diag"]:
                for a in (0, 1):
                    sl = exp_pair[:, a, ch["off"]:ch["off"] + 128]
                    nc.vector.tensor_mul(sl, sl, tri01)

    # ---- schedule ----
    for est, fn in proj_qk_units(0):
        fn()
    fillers.extend(proj_v_unit(j) for j in range(8))

    for hp in range(NPAIR):
        # queue next pair's projections; they MUST fully emit before that
        # pair's scores groups, so they are force-drained at iteration end
        proj_next = list(proj_qk_units(hp + 1)) if hp + 1 < NPAIR else []
        fillers.extend(proj_next)
        n_proj_next = len(proj_next)

        exp_pair = ex.tile([128, 2, exp_cols], BF16, tag="exp", name=f"exp{hp}")
        for gi, g in enumerate(groups):
            scores_group(hp, g, exp_pair)
            if hp == NPAIR - 1 and gi == len(groups) - 1:
                # final group: its own ctx c0 only needs earlier groups'
                # exp, so it overlaps the last Exp ACT
                for a in (0, 1):
                    _, fn = ctx_unit(2 * hp + a, exp_pair, 0)
                    fn()
            else:
                emit_fillers(2000)

        # force-drain queued proj/v units (later stages depend on them);
        # ctx units may linger as fillers for the next pair's scores
        keep = []
        for u in fillers:
            if u in proj_next or u[0] == 1100:  # proj or v units
                u[1]()
            else:
                keep.append(u)
        fillers[:] = keep

        if hp == NPAIR - 1:
            while fillers:
                est, fn = fillers.pop(0)
                fn()
            for a in (0, 1):
                _, fn = ctx_unit(2 * hp + a, exp_pair, 1)
                fn()
        else:
            for c in (0, 1):
                for a in (0, 1):
                    fillers.append(ctx_unit(2 * hp + a, exp_pair, c))


_PROGRAM = None
_PROGRAM_LOCK = threading.Lock()


def _get_program() -> bass.Bass:
    global _PROGRAM
    with _PROGRAM_LOCK:
        if _PROGRAM is None:
            nc = bacc.Bacc(None, target_bir_lowering=False)
            xT = nc.declare_dram_parameter("xT", [D, S], BF16, isOutput=False)
            wq = nc.declare_dram_parameter("wq", [D, HL * DH], BF16, isOutput=False)
            wk = nc.declare_dram_parameter("wk", [D, HL * DH], BF16, isOutput=False)
            wv = nc.declare_dram_parameter("wv", [D, HL * DH], BF16, isOutput=False)
            im = nc.declare_dram_parameter("im", [128, 2, 128], BF16, isOutput=False)
            y = nc.declare_dram_parameter("y_aug", [HL, DH + 1, S], F32, isOutput=True)
            with tile.TileContext(nc) as tc, ExitStack() as ctx:
                _emit_kernel(ctx, tc, xT, wq, wk, wv, im, y)
            nc.finalize()  # runs Bacc passes (reg alloc, wait splitting)
            _PROGRAM = nc
    return _PROGRAM


def make_in_maps(x, Wq, Wk, Wv):
    """Per-core input dicts: batch b=core//2, heads (core%2)*6..+6."""
    bf = ml_dtypes.bfloat16
    im = np.zeros((128, 2, 128), np.float32)
    im[:, 0, :] = np.eye(128)
    t = np.arange(128)
    im[:, 1, :] = (t[None, :] >= t[:, None]).astype(np.float32)
    im = im.astype(bf)
    in_maps = []
    for core in range(NCORES):
        b, hs = core // 2, (core % 2) * HL
        xTc = np.ascontiguousarray(np.asarray(x[b]).T.astype(bf))
        maps = {"xT": xTc, "im": im}
        for name, W in (("wq", Wq), ("wk", Wk), ("wv", Wv)):
            # [6,768,64] -> [768, 6*64], col = h*64+e
            maps[name] = np.ascontiguousarray(
                np.asarray(W[hs:hs + HL]).transpose(1, 0, 2)
                .reshape(D, HL * DH).astype(bf))
        in_maps.append(maps)
    return in_maps


def assemble_output(per_core_results):
    y_full = np.zeros((B, S, H * DH), np.float32)
    for core in range(NCORES):
        ya = per_core_results[core]["y_aug"]  # [6, 65, 1024]
        b, hs = core // 2, (core % 2) * HL
        ctxs = ya[:, 0:DH, :] / ya[:, DH:DH + 1, :]          # [6, 64, 1024]
        y_full[b, :, hs * DH:(hs + HL) * DH] = (
            ctxs.transpose(2, 0, 1).reshape(S, HL * DH))
    return y_full


def kernel(x, Wq, Wk, Wv):
    nc = _get_program()
    in_maps = make_in_maps(x, Wq, Wk, Wv)
    res = run_bass_kernel_spmd(nc, in_maps, core_ids=list(range(NCORES)))
    return assemble_output(res.results)



# revision 4
# speedup vs baseline: 1.0821x; 1.0821x over previous
"""Multi-head causal attention (B=4,S=1024,D=768,H=12,Dh=64) on 8 trn2 cores.

Sharding: core c handles batch b=c//2 and the 6 heads hs=(c%2)*6 .. hs+6
(head-axis tensor parallel x batch parallel; 8 cores = 4 batches x 2 head-halves).

Per-core on-chip dataflow (bf16 matmul operands, fp32 PSUM accumulation):
  xT [128,6,1024] (host-prepacked bf16), W{q,k,v} host-prepacked [128,6,384]
  qT/kT = W-chunk.T(lhsT) @ xT    -> [64,1024] per head (transposed layout)
  v     = xT-chunk.T @ Wv          -> [1024, 6*65] per t-chunk (65th col = ones)
  scoresT[t,s] computed per head-pair in 9 groups of 512 cols/head, each group
  a [128,2,512] PSUM tile (double-buffered so exp pipelines against matmuls);
  exp via ScalarE Exp(scale=1/8) from PSUM into a flat bf16 SBUF buffer; diag
  chunks masked afterwards on VectorE (multiply by the 0/1 triangle)
  ctxT_aug[65, s] = sum_j v_aug_j(lhsT) @ expT_j  (row 64 = softmax denominator)
  y_aug[h, 0:65, s] DMA'd out bf16; host divides by denominators + transposes.
"""

import threading
from contextlib import ExitStack

import ml_dtypes
import numpy as np

import concourse.bass as bass
import concourse.tile as tile
from concourse import bacc, mybir
from concourse.bass_utils import run_bass_kernel_spmd

B, S, D, H, DH = 4, 1024, 768, 12, 64
NCORES = 8
HL = H // 2          # 6 local heads per core
KC = D // 128        # 6 contraction chunks
NPAIR = HL // 2      # head pairs for qk projection
F32 = mybir.dt.float32
BF16 = mybir.dt.bfloat16


def _attn_groups():
    """Chunk table for one head's scoresT, packed into 9 [128,512] groups.

    A chunk (j, c) is the scoresT tile for t-chunk j (rows j*128..j*128+128)
    and s-range [s0, s0+w) inside output half c (s in [512c, 512c+512)).
    Only causal-relevant chunks exist; `diag` chunks get the triangular mask
    applied to their first 128 columns after exp. Each group holds exactly
    512 columns per head (one PSUM bank per head), so the group pool can be
    double-buffered and exp overlaps the next group's matmuls. Groups are
    ordered so half c=1 completes early (ctx c1 can run during later groups)
    and the last groups hold only c=0 chunks (short tail).
    """
    def chunk(j, c, ps_off):
        s0 = max(512 * c, 128 * j)
        w = 512 * (c + 1) - s0
        return dict(j=j, c=c, s0=s0, w=w, diag=(s0 == 128 * j), ps_off=ps_off)

    groups = [
        [chunk(0, 1, 0)],
        [chunk(1, 1, 0)],
        [chunk(2, 1, 0)],
        [chunk(3, 1, 0)],
        [chunk(4, 1, 0)],
        [chunk(5, 1, 0), chunk(7, 1, 384)],
        [chunk(6, 1, 0), chunk(2, 0, 256)],
        [chunk(0, 0, 0)],
        [chunk(1, 0, 0), chunk(3, 0, 384)],
    ]
    for gi, g in enumerate(groups):
        assert sum(ch["w"] for ch in g) == 512
        for ch in g:
            ch["off"] = 512 * gi + ch["ps_off"]
    return groups


def _emit_kernel(ctx: ExitStack, tc: tile.TileContext, xT, wq, wk, wv, im, y):
    nc = tc.nc
    groups = _attn_groups()

    # ---- pools ----
    const = ctx.enter_context(tc.tile_pool(name="const", bufs=1))
    xtw = ctx.enter_context(tc.tile_pool(name="xtw", bufs=1))
    qk_pool = ctx.enter_context(tc.tile_pool(name="qk", bufs=1))
    # PSUM budget: pj 2 banks + sg 2x2 + cx 2x1 = 8
    pj = ctx.enter_context(tc.tile_pool(name="pj", bufs=1, space="PSUM"))
    sg = ctx.enter_context(tc.tile_pool(name="sg", bufs=2, space="PSUM"))
    cx = ctx.enter_context(tc.tile_pool(name="cx", bufs=2, space="PSUM"))
    ex = ctx.enter_context(tc.tile_pool(name="ex", bufs=2))
    ysb = ctx.enter_context(tc.tile_pool(name="ysb", bufs=6))

    tri01 = const.tile([128, 128], BF16)   # 1 where s >= t else 0
    warm = const.tile([128, 512], BF16)

    xt = xtw.tile([128, KC, S], BF16)
    w_q = xtw.tile([128, KC, HL * DH], BF16)
    w_k = xtw.tile([128, KC, HL * DH], BF16)
    w_v = xtw.tile([128, KC, HL * DH], BF16)

    qT = qk_pool.tile([128, NPAIR, S], BF16)  # partitions: (h%2)*64+e, pair, s
    kT = qk_pool.tile([128, NPAIR, S], BF16)
    v_sb = qk_pool.tile([128, 8, HL * (DH + 1)], BF16)  # [t_rel, t_chunk, h*65+x]

    # ---- t=0: all input DMA kicks first (few, large, spread over queues).
    # sync (HWDGE): tri, xt in 3 chunks, then wv; scalar (HWDGE): wq, wk.
    # The host pre-packs every tensor into its exact SBUF layout so each
    # dma_start is one contiguous-per-partition transfer.
    nc.vector.memset(warm, 0.0)
    nc.sync.dma_start(out=tri01, in_=im[:, :])
    for i3 in range(3):
        nc.sync.dma_start(out=xt[:, 2 * i3:2 * i3 + 2, :],
                          in_=xT[:, 2 * i3:2 * i3 + 2, :])
    nc.scalar.dma_start(out=w_q, in_=wq[:, :, :])
    nc.scalar.dma_start(out=w_k, in_=wk[:, :, :])
    nc.sync.dma_start(out=w_v, in_=wv[:, :, :])

    # PE warm-up on the zero tile (no DMA dependency): ~3.4us of matmuls so
    # the HAM clock gate opens (K=8/8, 2.4 GHz) while the input DMAs land.
    warm_ps = pj.tile([128, 512], F32, tag="pjq0", name="warm")
    for i in range(8):
        nc.tensor.matmul(out=warm_ps, lhsT=warm[:, 0:128], rhs=warm,
                         start=(i == 0), stop=(i == 7))

    # ---- PE filler machinery: engines run their streams in order, so the
    # scores groups (paced by the Scalar-engine exp) need independent matmul
    # work interleaved into the PE stream to avoid idle gaps.
    fillers = []  # list of (est_ns, emit_fn, kind)

    def emit_fillers(budget_ns):
        while fillers and budget_ns > 0:
            est, fn, _ = fillers.pop(0)
            fn()
            budget_ns -= est

    def drain_fillers(kinds=None):
        keep = []
        for u in fillers:
            if kinds is None or u[2] in kinds:
                u[1]()
            else:
                keep.append(u)
        fillers[:] = keep

    def proj_qk_units(pp, kind):
        """q/k projection for pair pp as filler units (kc-outer accumulate,
        LDWEIGHTS shared between the two 512-col output banks)."""
        units = []
        for w_all, dst in ((w_q, qT), (w_k, kT)):
            pss = [pj.tile([128, 512], F32, tag=f"pjq{i}", name=f"ps{pp}{i}")
                   for i in range(2)]

            def unit(kcs, w_all=w_all, pss=pss, pp=pp, dst=dst):
                def emit():
                    for kc in kcs:
                        for i, ps in enumerate(pss):
                            nc.tensor.matmul(
                                out=ps,
                                lhsT=w_all[:, kc, pp * 128:(pp + 1) * 128],
                                rhs=xt[:, kc, i * 512:(i + 1) * 512],
                                start=(kc == 0), stop=(kc == KC - 1),
                            )
                    if kcs[-1] == KC - 1:
                        for i, ps in enumerate(pss):
                            nc.vector.tensor_copy(
                                out=dst[:, pp, i * 512:(i + 1) * 512], in_=ps)
                return emit
            units.append((900, unit([0, 1]), kind))
            units.append((900, unit([2, 3]), kind))
            units.append((900, unit([4, 5]), kind))
        return units

    def proj_v_unit(j):
        def emit():
            psv = pj.tile([128, HL * DH], F32, tag=f"pjq{j % 2}", name=f"psv{j}")
            for kc in range(KC):
                nc.tensor.matmul(
                    out=psv,
                    lhsT=xt[:, kc, j * 128:(j + 1) * 128],
                    rhs=w_v[:, kc, :],
                    start=(kc == 0), stop=(kc == KC - 1),
                )
            v_dst = v_sb[:, j, :].rearrange("p (h x) -> p h x", h=HL)
            nc.vector.tensor_copy(
                out=v_dst[:, :, 0:DH],
                in_=psv.rearrange("p (h e) -> p h e", h=HL),
            )
            nc.vector.memset(v_dst[:, :, DH:DH + 1], 1.0)
        return (1300, emit, "v")

    chunks = [ch for g in groups for ch in g]
    c1_chunks = sorted((ch for ch in chunks if ch["c"] == 1),
                       key=lambda t: t["j"])
    c0_chunks = sorted((ch for ch in chunks if ch["c"] == 0),
                       key=lambda t: t["j"])

    # per-(pair-index, head, half) state shared across ctx sub-units
    ctx_state = {}

    def ctx_subunit(hp, h, c, sub, exp_pair, last_evac_on_scalar=False):
        """Context sub-unit: accumulate a subset of half-c chunks for head h.

        sub selects chunks: c=1 -> 'a' = j0..4, 'b' = j5..7 (+evacuate);
        c=0 -> 'a' = j in (0,2), 'b' = j in (1,3) (+evacuate).
        Evacuation casts fp32 PSUM -> bf16 half of the per-head y tile; the
        c=0 evacuation (always last) also kicks the per-head output DMA.
        """
        if c == 1:
            cc = [ch for ch in c1_chunks
                  if (ch["j"] <= 4) == (sub == "a")]
        else:
            cc = [ch for ch in c0_chunks
                  if (ch["j"] in (0, 2)) == (sub == "a")]
        first = (sub == "a")
        last = (sub == "b")
        est = sum(ch["w"] for ch in cc) * 5 // 12 + (500 if last else 100)

        def emit():
            key = (hp, h, c)
            if first:
                ctx_state[key] = cx.tile([DH + 1, 512], F32, tag="cx",
                                         name=f"pc{hp}{h}{c}")
            pc = ctx_state[key]
            for idx, ch in enumerate(cc):
                nc.tensor.matmul(
                    out=pc[:, ch["s0"] - 512 * c: ch["s0"] - 512 * c + ch["w"]],
                    lhsT=v_sb[:, ch["j"], :].rearrange(
                        "p (hh x) -> p hh x", hh=HL)[:, h, :],
                    rhs=exp_pair[:, h % 2, ch["off"]:ch["off"] + ch["w"]],
                    start=(first and idx == 0), stop=(last and idx == len(cc) - 1),
                )
            if last:
                ykey = (hp, h)
                if ykey not in ctx_state:
                    ctx_state[ykey] = ysb.tile([DH + 1, S], BF16, tag="ysb",
                                               name=f"y{hp}{h}")
                yt = ctx_state[ykey]
                eng = nc.scalar if last_evac_on_scalar else nc.vector
                if last_evac_on_scalar:
                    eng.copy(out=yt[:, c * 512:(c + 1) * 512], in_=pc)
                else:
                    eng.tensor_copy(out=yt[:, c * 512:(c + 1) * 512], in_=pc)
                if c == 0:  # both halves done -> ship the head
                    nc.sync.dma_start(out=y[h, :, :], in_=yt)
        return (est, emit, "ctx")

    def ctx_units(hp, exp_pair, scalar_evac=False):
        us = []
        for c, sub in ((1, "a"), (1, "b"), (0, "a"), (0, "b")):
            for a in (0, 1):
                us.append(ctx_subunit(hp, 2 * hp + a, c, sub, exp_pair,
                                      last_evac_on_scalar=scalar_evac))
        return us

    def scores_group(hp, gi, exp_pair):
        """One scores group for both heads of pair hp into one [128,2,512]
        PSUM tile (head A bank 0, head B bank 1). A/B matmuls alternate so
        their K=64 row groups (base_partition 0/64) run concurrently. One
        Exp ACT covers both heads via the 3D AP; causal masking of diag
        chunks happens afterwards on the Vector engine."""
        g = groups[gi]
        ps = sg.tile([128, 2, 512], F32, tag="sg", name=f"sg{hp}_{gi}")
        for i, ch in enumerate(g):
            for a in (0, 1):
                nc.tensor.matmul(
                    out=ps[:, a, ch["ps_off"]:ch["ps_off"] + ch["w"]],
                    lhsT=kT[a * 64:a * 64 + 64, hp,
                            ch["j"] * 128:(ch["j"] + 1) * 128],
                    rhs=qT[a * 64:a * 64 + 64, hp,
                           ch["s0"]:ch["s0"] + ch["w"]],
                    start=(i == 0), stop=(i == len(g) - 1),
                )
        nc.scalar.activation(
            out=exp_pair[:, :, gi * 512:(gi + 1) * 512],
            in_=ps,
            func=mybir.ActivationFunctionType.Exp,
            scale=1.0 / np.sqrt(DH),
        )
        for ch in g:
            if ch["diag"]:
                for a in (0, 1):
                    sl = exp_pair[:, a, ch["off"]:ch["off"] + 128]
                    nc.vector.tensor_mul(sl, sl, tri01)

    # ---- schedule ----
    # lead-in: pair-0 q/k projections emitted directly
    for est, fn, _ in proj_qk_units(0, "proj"):
        fn()

    # fillers for pair 0: v projections interleaved with pair-1 projections
    p1 = proj_qk_units(1, "proj1")
    vu = [proj_v_unit(j) for j in range(8)]
    for i in range(6):
        fillers.append(vu[i])
        fillers.append(p1[i])
    fillers.extend(vu[6:])

    for hp in range(NPAIR):
        exp_pair = ex.tile([128, 2, 9 * 512], BF16, tag="exp", name=f"exp{hp}")
        lastp = hp == NPAIR - 1
        for gi in range(len(groups)):
            scores_group(hp, gi, exp_pair)
            if lastp:
                # feed the endgame: this pair's ctx sub-units become
                # available as their exp prefixes complete
                cu = ctx_units(hp, exp_pair, scalar_evac=True)
                if gi == 4:
                    fillers[0:0] = cu[0:2]       # c1a A,B
                elif gi == 6:
                    fillers[0:0] = cu[2:4]       # c1b A,B
                elif gi == 7:
                    fillers[0:0] = cu[4:6]       # c0a A,B
                elif gi == 8:
                    drain_fillers()
                    for est, fn, _ in cu[6:8]:   # c0b A,B + evac + DMA
                        fn()
                    continue
            emit_fillers(800)

        if hp == 0:
            # v and pair-1 proj must be done before pair-1 scores / ctx
            drain_fillers(("v", "proj1"))
            fillers.extend(proj_qk_units(2, "proj2"))
            fillers.extend(ctx_units(0, exp_pair))
        elif hp == 1:
            drain_fillers(("proj2", "ctx"))
            fillers.extend(ctx_units(1, exp_pair))


_PROGRAM = None
_PROGRAM_LOCK = threading.Lock()


def _get_program() -> bass.Bass:
    global _PROGRAM
    with _PROGRAM_LOCK:
        if _PROGRAM is None:
            nc = bacc.Bacc(None, target_bir_lowering=False)
            xT = nc.declare_dram_parameter("xT", [128, KC, S], BF16, isOutput=False)
            wq = nc.declare_dram_parameter("wq", [128, KC, HL * DH], BF16, isOutput=False)
            wk = nc.declare_dram_parameter("wk", [128, KC, HL * DH], BF16, isOutput=False)
            wv = nc.declare_dram_parameter("wv", [128, KC, HL * DH], BF16, isOutput=False)
            im = nc.declare_dram_parameter("im", [128, 128], BF16, isOutput=False)
            y = nc.declare_dram_parameter("y_aug", [HL, DH + 1, S], BF16, isOutput=True)
            with tile.TileContext(nc) as tc, ExitStack() as ctx:
                _emit_kernel(ctx, tc, xT, wq, wk, wv, im, y)
            nc.finalize()  # runs Bacc passes (reg alloc, wait splitting)
            _PROGRAM = nc
    return _PROGRAM


def make_in_maps(x, Wq, Wk, Wv):
    """Per-core input dicts: batch b=core//2, heads (core%2)*6..+6.

    Every tensor is pre-packed into its SBUF layout (partition dim first)
    so each on-device dma_start is a single contiguous transfer.
    """
    bf = ml_dtypes.bfloat16
    t = np.arange(128)
    im = (t[None, :] >= t[:, None]).astype(bf)  # 1 where s >= t
    in_maps = []
    for core in range(NCORES):
        b, hs = core // 2, (core % 2) * HL
        xTc = np.asarray(x[b]).T.astype(bf)              # [768, 1024]
        xTc = np.ascontiguousarray(
            xTc.reshape(KC, 128, S).transpose(1, 0, 2))  # [128, 6, 1024]
        maps = {"xT": xTc, "im": im}
        for name, W in (("wq", Wq), ("wk", Wk), ("wv", Wv)):
            # [6,768,64] -> [768, 6*64] (col = h*64+e) -> [128, 6, 384]
            wf = np.asarray(W[hs:hs + HL]).transpose(1, 0, 2).reshape(D, HL * DH)
            maps[name] = np.ascontiguousarray(
                wf.reshape(KC, 128, HL * DH).transpose(1, 0, 2).astype(bf))
        in_maps.append(maps)
    return in_maps


def assemble_output(per_core_results):
    y_full = np.zeros((B, S, H * DH), np.float32)
    for core in range(NCORES):
        ya = per_core_results[core]["y_aug"].astype(np.float32)  # [6, 65, 1024]
        b, hs = core // 2, (core % 2) * HL
        ctxs = ya[:, 0:DH, :] / ya[:, DH:DH + 1, :]              # [6, 64, 1024]
        y_full[b, :, hs * DH:(hs + HL) * DH] = (
            ctxs.transpose(2, 0, 1).reshape(S, HL * DH))
    return y_full


def kernel(x, Wq, Wk, Wv):
    nc = _get_program()
    in_maps = make_in_maps(x, Wq, Wk, Wv)
    res = run_bass_kernel_spmd(nc, in_maps, core_ids=list(range(NCORES)))
    return assemble_output(res.results)
